# revision 8
# baseline (speedup 1.0000x reference)
"""Bi-directional MinGRU kernel for Trainium2 (8 NeuronCores, SPMD).

Problem: x [4, 4096, 1024]; per direction d in {fwd, bwd}:
    k  = x @ Wz_d + bz_d
    A  = sigmoid(-k)           (= 1 - z, the carry coefficient)
    z  = sigmoid(k)
    gp = x @ Wh_d + bh_d
    g  = max(gp + 0.5, sigmoid(gp))      (== where(gp>=0, gp+0.5, sigmoid(gp)))
    h_t = A_t * h_{t-1} + z_t * g_t      (linear first-order scan over S)
    out = concat(h_fwd, h_bwd) @ W_out + b_out

Sharding: 8 cores = (4 batches) x (2 directions). Each core computes the
full hidden state for one (batch, direction) and its half of the final
2H->H projection; the two partial products per batch are summed on host.

Per-core layout: everything is kept transposed ([channel, seq]) so the
sequential scan runs along the free dimension with channels on partitions,
using the native VectorE tensor_tensor_scan instruction.

Mixed precision: the K-GEMM (gate pre-activation k) runs entirely in
fp8-e4m3 with perf_mode=DoubleRow (2 contraction slabs per matmul, 0.5
cyc/row): its error is damped ~4x by the sigmoids. The G-GEMM runs
N_G_F8 of its 8 contraction slabs in fp8-DR (the g path passes error
1:1 where gp>0, so only a fraction fits in the error budget). The
O-GEMM stays bf16 (direct output path, most sensitive). fp8 operands
are pre-scaled on host (x*32, W*1024 -> PSUM = 2^15 * logical value,
clipped to +-240 = TRN FP8_EXP4 max normal); the 2^-15 descale folds
into the ACT scale ports. bf16 G-slabs' weights are pre-scaled 2^15 so
both halves accumulate into PSUM at the same scale (power-of-2 scaling
is exact in bf16).

Measured (2026-08-08): rel 1.597e-2 (sim predicted 1.606e-2 — DR is
bit-correct on HW). HW exec 340.3us, but the WHOLE CORE clocked at
~2.0GHz instead of ~2.4GHz during the run (all engines' op durations
uniformly 1.2x the baseline run's) — investigating whether that's
kernel-induced (power) or environment drift.
"""

import os
import numpy as np
from contextlib import ExitStack

import concourse.bass as bass
import concourse.tile as tile
from concourse import bacc, mybir
from concourse.bass_utils import run_bass_kernel_spmd

P = 128          # partitions
S = 4096         # sequence length
D = 1024         # input dim
H = 1024         # hidden dim
SC = 512         # seq chunk (one PSUM bank of fp32)
NSC = S // SC    # 8 seq chunks
ND = D // P      # 8 contraction tiles for GEMM1
NH = H // P      # 8 hidden tiles
NCORES = 8

F32 = mybir.dt.float32
BF16 = mybir.dt.bfloat16
F8 = mybir.dt.float8e4

# fp8 slab counts (build-time constants): K-GEMM all-fp8, G-GEMM N_G_F8
# of 8 slabs fp8 (rest bf16) on the first G_F8_CHUNKS chunks only, O-GEMM
# bf16. G_F8_CHUNKS trades precision for DR-matmul density: the SoC power
# manager drops the whole-core clock 2.4->2.0GHz when the average DR
# density crosses a threshold somewhere in (20%, 26.5%) of matmul count
# (measured: K-only = 256 DR of 1288 runs at full clock; K+G2-all-chunks
# = 320 of 1209 throttles, erasing the fp8 gain).
K_F8 = os.environ.get("BIMINGRU_K_F8", "1") == "1"
N_G_F8 = int(os.environ.get("BIMINGRU_G_F8", "2"))
G_F8_CHUNKS = int(os.environ.get("BIMINGRU_G_F8_CHUNKS", "4"))
if N_G_F8 == 0:
    G_F8_CHUNKS = 0
if G_F8_CHUNKS == 0:
    N_G_F8 = 0
assert N_G_F8 % 2 == 0 and 0 <= N_G_F8 <= ND
NGB = ND - N_G_F8                # bf16 G slabs on fp8-G chunks

SX = 32.0                        # host scale on fp8 x
SW = 1024.0                      # host scale on fp8 W
PS = SX * SW                     # PSUM scale of fp8-fed GEMMs (2^15)
KS = (1.0 / PS) if K_F8 else 1.0     # K descale folded into ACT scale
GS = (1.0 / PS) if N_G_F8 else 1.0   # G descale

DR = mybir.MatmulPerfMode.DoubleRow


def _np_f8():
    import ml_dtypes
    return np.dtype(ml_dtypes.float8_e4m3)


def _np_bf16():
    import ml_dtypes
    return np.dtype(ml_dtypes.bfloat16)


def _build_module():
    nc = bacc.Bacc("TRN2", target_bir_lowering=False, debug=False)

    # All inputs are host-blocked so every SBUF working set is ONE contiguous
    # DMA (the sync engine's ~0.65us per-DMA issue cost dominates the ramp):
    #   xT8 [NSC*P, ND, SC]: [j*128+p, d, c] = 32*x[j*512+c, d*128+p] (fp8)
    #   xTb [NSC*P, NGB, SC]: same but bf16, slabs d >= N_G_F8 only, unscaled
    #   Wz8 [NH*P, ND, P]:  [i*128+p, d, c] = 1024*Wz[d*128+p, i*128+c] (fp8)
    #   Wh8 [NH*P, N_G_F8, P]: fp8 slabs of Wh (scaled 1024)
    #   Whb [NH*P, NGB, P]: bf16 slabs of Wh (scaled 2^15 iff N_G_F8>0)
    #   Wo  [H, H] bf16: [o*128+p, i*128+c] = W_half[i*128+p, o*128+c]
    #   biasT [128, 4*NH] = [bz | -bz | bh | bh+0.5] per-partition columns
    x8 = nc.dram_tensor("x8", [NSC * P, ND, SC], F8, kind="ExternalInput").ap()
    if K_F8:
        Wz = nc.dram_tensor("Wz", [NH * P, ND, P], F8, kind="ExternalInput").ap()
    else:
        Wz = nc.dram_tensor("Wz", [NH * P, ND, P], BF16, kind="ExternalInput").ap()
    if N_G_F8:
        Wh8 = nc.dram_tensor("Wh8", [NH * P, N_G_F8, P], F8,
                             kind="ExternalInput").ap()
    xb = nc.dram_tensor("xb", [NSC * P, ND, SC], BF16, kind="ExternalInput").ap()
    Whb = nc.dram_tensor("Whb", [NH * P, ND, P], BF16, kind="ExternalInput").ap()
    Wo = nc.dram_tensor("Wo", [H, H], BF16, kind="ExternalInput").ap()
    biasT = nc.dram_tensor("biasT", [P, 4 * NH], F32, kind="ExternalInput").ap()
    outT = nc.dram_tensor("outT", [H, S], F32, kind="ExternalOutput").ap()

    AF = mybir.ActivationFunctionType
    OP = mybir.AluOpType

    with tile.TileContext(nc) as tc, ExitStack() as ctx:
        wpool = ctx.enter_context(tc.tile_pool(name="w", bufs=1))
        xpool = ctx.enter_context(tc.tile_pool(name="x", bufs=2))
        pspool = ctx.enter_context(tc.tile_pool(name="ps", bufs=2, space="PSUM"))
        ewpool = ctx.enter_context(tc.tile_pool(name="ew", bufs=2))
        hpool = ctx.enter_context(tc.tile_pool(name="h", bufs=2))
        opool = ctx.enter_context(tc.tile_pool(name="o", bufs=3))

        # --- PE warm-up: the first real matmul can't start until the head of
        # the input DMA stream lands, and a cold PE then runs at 1.2GHz for
        # another ~3.4us (HAM). Burn that idle window with dummy matmuls on
        # memset-zero tiles so the HAM un-throttles before real work
        # arrives. The dummy PSUM tile reuses the psK tag (no extra bank).
        wdum = ewpool.tile([P, P], BF16, tag="wdum", name="wdum")
        nc.vector.memset(wdum[:], 0)
        rdum = ewpool.tile([P, SC], BF16, tag="rdum", name="rdum")
        nc.vector.memset(rdum[:], 0)
        # 5 dummies: enough to keep the PE busy from the end of the preamble
        # (~8.1us) until the first real operands land (~9.5us); the real MM
        # stream then continues the HAM warmup window.
        psdum = pspool.tile([P, SC], F32, tag="psK", bufs=3, name="psdum")
        for _ in range(5):
            nc.tensor.matmul(psdum[:], wdum[:], rdum[:], start=True, stop=True)

        x8_chunks = {}
        xb_chunks = {}

        def load_x_chunk(j):
            # one DMA per chunk & dtype: fp8 [128, ND, SC], bf16 [128, ND, SC].
            # fp8-G chunks never read bf16 slabs < N_G_F8, so skip them.
            xt = xpool.tile([P, ND, SC], F8, tag="x8b", name=f"x8b_{j}")
            nc.sync.dma_start(xt[:], x8[j * P:(j + 1) * P, :, :])
            x8_chunks[j] = xt
            xtb = xpool.tile([P, ND, SC], BF16, tag="xbb", name=f"xbb_{j}")
            d0 = N_G_F8 if j < G_F8_CHUNKS else 0
            nc.sync.dma_start(xtb[:, d0:, :], xb[j * P:(j + 1) * P, d0:, :])
            xb_chunks[j] = xtb

        # Startup: x chunk 0 is on the critical path to the first matmul, so
        # split it into 4 slab-pair DMAs (parallel queues + per-MM wait
        # granularity); then the i-blocked Wz/Wh tiles interleaved — K(0,i)
        # unblocks as soon as Wz[i] lands, so the PE ramps with the DMA
        # stream.
        Wz_t, Wh8_t, Whb_t, Wo_t = [], [], [], []
        xt0 = xpool.tile([P, ND, SC], F8, tag="x8b", name="x8b_0")
        nc.sync.dma_start(xt0[:, 0:2, :], x8[0:P, 0:2, :])
        wzt = wpool.tile([P, ND, P], Wz.dtype, tag="wz0", name="wz0")
        nc.sync.dma_start(wzt[:], Wz[0:P, :, :])
        Wz_t.append(wzt)
        for q in range(1, 4):
            nc.sync.dma_start(xt0[:, 2 * q:2 * q + 2, :],
                              x8[0:P, 2 * q:2 * q + 2, :])
        x8_chunks[0] = xt0

        bias_sb = wpool.tile([P, 4 * NH], F32, tag="bias", name="bias_sb")
        nc.sync.dma_start(bias_sb[:], biasT[:, :])
        bz_sb = bias_sb[:, 0:NH]
        nbz_sb = bias_sb[:, NH:2 * NH]
        bh_sb = bias_sb[:, 2 * NH:3 * NH]
        bh5_sb = bias_sb[:, 3 * NH:4 * NH]

        # G(0,0)'s operands next: Wh tiles for i=0, then chunk-0 bf16 x in
        # slab-pair DMAs so each G(0,0) matmul unblocks as its pair lands
        # (a single 1MB xtb0 DMA measured a 5.3us PE stall at t~14.5us).
        wzt = wpool.tile([P, ND, P], Wz.dtype, tag="wz1", name="wz1")
        nc.sync.dma_start(wzt[:], Wz[P:2 * P, :, :])
        Wz_t.append(wzt)

        def load_wh(i):
            if N_G_F8:
                w8 = wpool.tile([P, N_G_F8, P], F8, tag=f"wh8_{i}",
                                name=f"wh8_{i}")
                nc.sync.dma_start(w8[:], Wh8[i * P:(i + 1) * P, :, :])
                Wh8_t.append(w8)
            wbt = wpool.tile([P, ND, P], BF16, tag=f"whb_{i}", name=f"whb_{i}")
            nc.sync.dma_start(wbt[:], Whb[i * P:(i + 1) * P, :, :])
            Whb_t.append(wbt)

        load_wh(0)
        xtb0 = xpool.tile([P, ND, SC], BF16, tag="xbb", name="xbb_0")
        d0 = N_G_F8 if 0 < G_F8_CHUNKS else 0
        for dp in range(d0, ND, 2):
            nc.sync.dma_start(xtb0[:, dp:dp + 2, :], xb[0:P, dp:dp + 2, :])
        xb_chunks[0] = xtb0
        for i in range(2, NH):
            wzt = wpool.tile([P, ND, P], Wz.dtype, tag=f"wz{i}", name=f"wz{i}")
            nc.sync.dma_start(wzt[:], Wz[i * P:(i + 1) * P, :, :])
            Wz_t.append(wzt)
            load_wh(i - 1)
        load_wh(NH - 1)

        def load_wo():
            for o in range(NH):
                wot = wpool.tile([P, H], BF16, tag=f"wo{o}", name=f"wo{o}")
                nc.sync.dma_start(wot[:], Wo[o * P:(o + 1) * P, :])
                Wo_t.append(wot)

        h_tiles = [[None] * NH for _ in range(NSC)]

        stash = {}

        def emit_k(j, i):
            xc = x8_chunks[j]
            psK = pspool.tile([P, SC], F32, tag="psK", bufs=3,
                              name=f"psK_{j}_{i}")
            if K_F8:
                for dp in range(ND // 2):
                    nc.tensor.matmul(
                        psK[:], Wz_t[i][:, 2 * dp:2 * dp + 2, :],
                        xc[:, 2 * dp:2 * dp + 2, :],
                        start=(dp == 0), stop=(dp == ND // 2 - 1),
                        perf_mode=DR)
            else:
                for d in range(ND):
                    nc.tensor.matmul(
                        psK[:], Wz_t[i][:, d:d + 1, :], xc[:, d:d + 1, :],
                        start=(d == 0), stop=(d == ND - 1))
            A = ewpool.tile([P, SC], F32, tag="A", bufs=3, name=f"A_{j}_{i}")
            nc.scalar.activation(A[:], psK[:], AF.Sigmoid,
                                 bias=nbz_sb[:, i:i + 1], scale=-KS)
            z = ewpool.tile([P, SC], F32, tag="z", bufs=3, name=f"z_{j}_{i}")
            nc.scalar.activation(z[:], psK[:], AF.Sigmoid,
                                 bias=bz_sb[:, i:i + 1], scale=KS)
            stash[(j, i)] = (A, z)

        def emit_g(j, i):
            psG = pspool.tile([P, SC], F32, tag="psG", bufs=3,
                              name=f"psG_{j}_{i}")
            fp8j = j < G_F8_CHUNKS
            if fp8j:
                xc8 = x8_chunks[j]
                for dp in range(N_G_F8 // 2):
                    nc.tensor.matmul(
                        psG[:], Wh8_t[i][:, 2 * dp:2 * dp + 2, :],
                        xc8[:, 2 * dp:2 * dp + 2, :],
                        start=(dp == 0), stop=False, perf_mode=DR)
            xcb = xb_chunks[j]
            d0 = N_G_F8 if fp8j else 0
            for d in range(d0, ND):
                nc.tensor.matmul(
                    psG[:], Whb_t[i][:, d:d + 1, :], xcb[:, d:d + 1, :],
                    start=(d == 0 and not fp8j), stop=(d == ND - 1))
            A, z = stash.pop((j, i))
            sg = ewpool.tile([P, SC], F32, tag="sg", name=f"sg_{j}_{i}")
            nc.scalar.activation(sg[:], psG[:], AF.Sigmoid,
                                 bias=bh_sb[:, i:i + 1], scale=GS)
            g = ewpool.tile([P, SC], F32, tag="g", name=f"g_{j}_{i}")
            if N_G_F8:
                # linear branch needs the 2^-15 descale before +bh5: one
                # extra ACT op (Identity has working bias+scale ports), then
                # the max moves to a plain DVE tensor_tensor.
                t = ewpool.tile([P, SC], F32, tag="t", name=f"t_{j}_{i}")
                nc.scalar.activation(t[:], psG[:], AF.Identity,
                                     bias=bh5_sb[:, i:i + 1], scale=GS)
                nc.vector.tensor_tensor(g[:], t[:], sg[:], op=OP.max)
            else:
                nc.vector.scalar_tensor_tensor(g[:], psG[:], bh5_sb[:, i:i + 1],
                                               sg[:], op0=OP.add, op1=OP.max)
            Bv = ewpool.tile([P, SC], F32, tag="B", name=f"B_{j}_{i}")
            nc.vector.tensor_tensor(Bv[:], z[:], g[:], op=OP.mult)

            ht = hpool.tile([P, SC], BF16, tag=f"h{i}", name=f"h_{j}_{i}")
            init = 0.0 if j == 0 else h_tiles[j - 1][i][:, SC - 1:SC]
            nc.vector.tensor_tensor_scan(ht[:], A[:], Bv[:], initial=init,
                                         op0=OP.mult, op1=OP.add)
            h_tiles[j][i] = ht

        def emit_o(j, o):
            psO = pspool.tile([P, SC], F32, tag="psO", name=f"psO_{j}_{o}")
            for i in range(NH):
                nc.tensor.matmul(
                    psO[:], Wo_t[o][:, i * P:(i + 1) * P],
                    h_tiles[j][i][:],
                    start=(i == 0), stop=(i == NH - 1))
            oc = opool.tile([P, SC], F32, tag="oc", name=f"oc_{j}_{o}")
            nc.scalar.copy(oc[:], psO[:])
            nc.sync.dma_start(outT[o * P:(o + 1) * P, j * SC:(j + 1) * SC], oc[:])

        # Software pipeline. Per chunk j the PE group order is
        #   K0 K1 [G0 O0] [K2 G1 O1] [K3 G2 O2] ... [K7 G6 O6] [G7 O7]
        # where O* are the GEMM3 groups of chunk j-1. Interleaving the O
        # groups keeps ~2 PE groups between G(i) and the DVE/ACT chain that
        # releases its PSUM bank, so the PE never stalls on the elementwise
        # tail. x(j+1) is prefetched at the head of chunk j; Wo loads are
        # issued at the head of chunk 1 (first needed by GEMM3 of chunk 0).
        for j in range(NSC):
            if j + 1 < NSC:
                load_x_chunk(j + 1)
            if j == 1:
                load_wo()
            # chunk 0 leads with 4 K groups (vs 2): buys the DMA stream an
            # extra ~1.7us before G(0,0)'s weights/x are needed
            lead = 4 if j == 0 else 2
            for i in range(lead):
                emit_k(j, i)
            for i in range(NH):
                if i + lead < NH:
                    emit_k(j, i + lead)
                emit_g(j, i)
                if j >= 1:
                    emit_o(j - 1, i)
        for o in range(NH - 1):
            emit_o(NSC - 1, o)
        # final O group split into two N=256 halves so the first half's
        # copy+store overlaps the second half's matmuls (shorter serial
        # tail before the drain barrier); PSUM/SBUF tags are reused so no
        # extra banks are allocated
        j, o = NSC - 1, NH - 1
        HC = SC // 2
        for half in range(2):
            psO = pspool.tile([P, HC], F32, tag="psO", name=f"psOt_{half}")
            for i in range(NH):
                nc.tensor.matmul(
                    psO[:], Wo_t[o][:, i * P:(i + 1) * P],
                    h_tiles[j][i][:, half * HC:(half + 1) * HC],
                    start=(i == 0), stop=(i == NH - 1))
            oc = opool.tile([P, HC], F32, tag="oc", name=f"oct_{half}")
            nc.scalar.copy(oc[:], psO[:])
            nc.sync.dma_start(
                outT[o * P:(o + 1) * P,
                     j * SC + half * HC:j * SC + (half + 1) * HC], oc[:])

    nc.compile()
    return nc


_CACHE = {}


def _get_module():
    if "nc" not in _CACHE:
        _CACHE["nc"] = _build_module()
    return _CACHE["nc"]


def _make_in_maps(x, Wz_f, bz_f, Wh_f, bh_f, Wz_b, bz_b, Wh_b, bh_b, W_out, b_out):
    np_f8 = _np_f8()
    np_bf = _np_bf16()
    f32 = np.float32

    def q8(a):
        # host fp8-e4m3 quantization; TRN FP8_EXP4 max normal is 240
        return np.clip(a, -240.0, 240.0).astype(np_f8)

    def blk_w(w):
        # [D, H] -> blocked [H, ND, P]: out[i*128+p, d, c] = w[d*128+p, i*128+c]
        w = np.asarray(w, dtype=f32).reshape(ND, P, NH, P)
        return np.ascontiguousarray(w.transpose(2, 1, 0, 3).reshape(H, ND, P))

    def blk_x(xs, rev):
        # [S, D] -> blocked [NSC*P, ND, SC]: out[j*128+p, d, c] = x[j*512+c, d*128+p]
        if rev:
            xs = xs[::-1]
        xs = xs.reshape(NSC, SC, ND, P)
        return np.ascontiguousarray(
            xs.transpose(0, 3, 2, 1).reshape(NSC * P, ND, SC))

    x = np.asarray(x, dtype=f32)
    W_out = np.asarray(W_out)

    def w_maps(Wz, Wh, W_half):
        m = {}
        wzb = blk_w(np.asarray(Wz, f32))
        m["Wz"] = q8(wzb * SW) if K_F8 else wzb.astype(np_bf)
        whb = blk_w(np.asarray(Wh, f32))
        if N_G_F8:
            m["Wh8"] = q8(whb[:, :N_G_F8] * SW)
        m["Whb"] = np.ascontiguousarray(whb * (PS if N_G_F8 else 1.0)
                                        ).astype(np_bf)
        wo = np.asarray(W_half, f32).reshape(NH, P, NH, P)
        m["Wo"] = np.ascontiguousarray(
            wo.transpose(2, 1, 0, 3).reshape(H, H)).astype(np_bf)
        return m

    wm_f = w_maps(Wz_f, Wh_f, W_out[:H])
    wm_b = w_maps(Wz_b, Wh_b, W_out[H:])

    def bias_pack(b_z, b_h):
        def col(v):  # [H] -> [128, NH] with col i = h-tile i
            return np.asarray(v, dtype=f32).reshape(NH, P).T
        b_z = np.asarray(b_z, dtype=f32)
        b_h = np.asarray(b_h, dtype=f32)
        return np.ascontiguousarray(np.concatenate(
            [col(b_z), col(-b_z), col(b_h), col(b_h + 0.5)], axis=1))

    bias_f = bias_pack(bz_f, bh_f)
    bias_b = bias_pack(bz_b, bh_b)

    in_maps = []
    for b in range(4):
        for rev, wm, bm in ((False, wm_f, bias_f), (True, wm_b, bias_b)):
            xblk = blk_x(x[b], rev=rev)
            in_maps.append({
                "x8": q8(xblk * SX),
                "xb": xblk.astype(np_bf),
                "biasT": bm, **wm})
    return in_maps


def _assemble(results, b_out):
    out = np.empty((4, S, H), np.float32)
    for b in range(4):
        out[b] = results[2 * b]["outT"].T
        out[b] += results[2 * b + 1]["outT"].T
    out += np.asarray(b_out, dtype=np.float32)
    return out


def kernel(x, Wz_f, bz_f, Wh_f, bh_f, Wz_b, bz_b, Wh_b, bh_b, W_out, b_out):
    nc = _get_module()
    in_maps = _make_in_maps(x, Wz_f, bz_f, Wh_f, bh_f,
                            Wz_b, bz_b, Wh_b, bh_b, W_out, b_out)
    res = run_bass_kernel_spmd(nc, in_maps, core_ids=list(range(NCORES)))
    return _assemble(res.results, b_out)


# revision 9
# speedup vs baseline: 1.1968x; 1.1968x over previous
"""Bi-directional MinGRU kernel for Trainium2 (8 NeuronCores, SPMD).

Problem: x [4, 4096, 1024]; per direction d in {fwd, bwd}:
    k  = x @ Wz_d + bz_d
    A  = sigmoid(-k)           (= 1 - z, the carry coefficient)
    z  = sigmoid(k)
    gp = x @ Wh_d + bh_d
    g  = max(gp + 0.5, sigmoid(gp))      (== where(gp>=0, gp+0.5, sigmoid(gp)))
    h_t = A_t * h_{t-1} + z_t * g_t      (linear first-order scan over S)
    out = concat(h_fwd, h_bwd) @ W_out + b_out

Sharding: 8 cores = (4 batches) x (2 directions). Each core computes the
full hidden state for one (batch, direction) and its half of the final
2H->H projection; the two partial products per batch are summed on host.

Per-core layout: everything is kept transposed ([channel, seq]) so the
sequential scan runs along the free dimension with channels on partitions,
using the native VectorE tensor_tensor_scan instruction.

Mixed precision: the K-GEMM (gate pre-activation k) runs entirely in
fp8-e4m3 with perf_mode=DoubleRow (2 contraction slabs per matmul, 0.5
cyc/row): its error is damped ~4x by the sigmoids. The G-GEMM runs
N_G_F8 of its 8 contraction slabs in fp8-DR (the g path passes error
1:1 where gp>0, so only a fraction fits in the error budget). The
O-GEMM stays bf16 (direct output path, most sensitive). fp8 operands
are pre-scaled on host (x*32, W*1024 -> PSUM = 2^15 * logical value,
clipped to +-240 = TRN FP8_EXP4 max normal); the 2^-15 descale folds
into the ACT scale ports. bf16 G-slabs' weights are pre-scaled 2^15 so
both halves accumulate into PSUM at the same scale (power-of-2 scaling
is exact in bf16).

Measured (2026-08-08): rel 1.597e-2 (sim predicted 1.606e-2 — DR is
bit-correct on HW). HW exec 340.3us, but the WHOLE CORE clocked at
~2.0GHz instead of ~2.4GHz during the run (all engines' op durations
uniformly 1.2x the baseline run's) — investigating whether that's
kernel-induced (power) or environment drift.
"""

import os
import numpy as np
from contextlib import ExitStack

import concourse.bass as bass
import concourse.tile as tile
from concourse import bacc, mybir
from concourse.bass_utils import run_bass_kernel_spmd

P = 128          # partitions
S = 4096         # sequence length
D = 1024         # input dim
H = 1024         # hidden dim
SC = 512         # seq chunk (one PSUM bank of fp32)
NSC = S // SC    # 8 seq chunks
ND = D // P      # 8 contraction tiles for GEMM1
NH = H // P      # 8 hidden tiles
NCORES = 8

F32 = mybir.dt.float32
BF16 = mybir.dt.bfloat16
F8 = mybir.dt.float8e4

# fp8 slab counts (build-time constants): K-GEMM all-fp8, G-GEMM N_G_F8
# of 8 slabs fp8 (rest bf16) on the first G_F8_CHUNKS chunks only, O-GEMM
# bf16. G_F8_CHUNKS trades precision for DR-matmul density: the SoC power
# manager drops the whole-core clock 2.4->2.0GHz when the average DR
# density crosses a threshold somewhere in (20%, 26.5%) of matmul count
# (measured: K-only = 256 DR of 1288 runs at full clock; K+G2-all-chunks
# = 320 of 1209 throttles, erasing the fp8 gain).
K_F8 = os.environ.get("BIMINGRU_K_F8", "1") == "1"
N_G_F8 = int(os.environ.get("BIMINGRU_G_F8", "2"))
# 6 chunks = 304 DR of ~1245 matmuls = 24.4% DR density: the highest
# measured config that holds the full 2.4GHz clock (312/1237 = 25.2%
# throttles to 2.0GHz; the threshold sits at 25%).
G_F8_CHUNKS = int(os.environ.get("BIMINGRU_G_F8_CHUNKS", "6"))
if N_G_F8 == 0:
    G_F8_CHUNKS = 0
if G_F8_CHUNKS == 0:
    N_G_F8 = 0
assert N_G_F8 % 2 == 0 and 0 <= N_G_F8 <= ND
NGB = ND - N_G_F8                # bf16 G slabs on fp8-G chunks

SX = 32.0                        # host scale on fp8 x
SW = 1024.0                      # host scale on fp8 W
PS = SX * SW                     # PSUM scale of fp8-fed GEMMs (2^15)
KS = (1.0 / PS) if K_F8 else 1.0     # K descale folded into ACT scale
GS = (1.0 / PS) if N_G_F8 else 1.0   # G descale

DR = mybir.MatmulPerfMode.DoubleRow


def _np_f8():
    import ml_dtypes
    return np.dtype(ml_dtypes.float8_e4m3)


def _np_bf16():
    import ml_dtypes
    return np.dtype(ml_dtypes.bfloat16)


def _build_module():
    nc = bacc.Bacc("TRN2", target_bir_lowering=False, debug=False)

    # All inputs are host-blocked so every SBUF working set is ONE contiguous
    # DMA (the sync engine's ~0.65us per-DMA issue cost dominates the ramp):
    #   xT8 [NSC*P, ND, SC]: [j*128+p, d, c] = 32*x[j*512+c, d*128+p] (fp8)
    #   xTb [NSC*P, NGB, SC]: same but bf16, slabs d >= N_G_F8 only, unscaled
    #   Wz8 [NH*P, ND, P]:  [i*128+p, d, c] = 1024*Wz[d*128+p, i*128+c] (fp8)
    #   Wh8 [NH*P, N_G_F8, P]: fp8 slabs of Wh (scaled 1024)
    #   Whb [NH*P, NGB, P]: bf16 slabs of Wh (scaled 2^15 iff N_G_F8>0)
    #   Wo  [H, H] bf16: [o*128+p, i*128+c] = W_half[i*128+p, o*128+c]
    #   biasT [128, 4*NH] = [bz | -bz | bh | bh+0.5] per-partition columns
    x8 = nc.dram_tensor("x8", [NSC * P, ND, SC], F8, kind="ExternalInput").ap()
    if K_F8:
        Wz = nc.dram_tensor("Wz", [NH * P, ND, P], F8, kind="ExternalInput").ap()
    else:
        Wz = nc.dram_tensor("Wz", [NH * P, ND, P], BF16, kind="ExternalInput").ap()
    if N_G_F8:
        Wh8 = nc.dram_tensor("Wh8", [NH * P, N_G_F8, P], F8,
                             kind="ExternalInput").ap()
    xb = nc.dram_tensor("xb", [NSC * P, ND, SC], BF16, kind="ExternalInput").ap()
    Whb = nc.dram_tensor("Whb", [NH * P, ND, P], BF16, kind="ExternalInput").ap()
    Wo = nc.dram_tensor("Wo", [H, H], BF16, kind="ExternalInput").ap()
    biasT = nc.dram_tensor("biasT", [P, 4 * NH], F32, kind="ExternalInput").ap()
    outT = nc.dram_tensor("outT", [H, S], F32, kind="ExternalOutput").ap()

    AF = mybir.ActivationFunctionType
    OP = mybir.AluOpType

    with tile.TileContext(nc) as tc, ExitStack() as ctx:
        wpool = ctx.enter_context(tc.tile_pool(name="w", bufs=1))
        xpool = ctx.enter_context(tc.tile_pool(name="x", bufs=2))
        pspool = ctx.enter_context(tc.tile_pool(name="ps", bufs=2, space="PSUM"))
        ewpool = ctx.enter_context(tc.tile_pool(name="ew", bufs=2))
        hpool = ctx.enter_context(tc.tile_pool(name="h", bufs=2))
        opool = ctx.enter_context(tc.tile_pool(name="o", bufs=3))

        # --- PE warm-up: the first real matmul can't start until the head of
        # the input DMA stream lands, and a cold PE then runs at 1.2GHz for
        # another ~3.4us (HAM). Burn that idle window with dummy matmuls on
        # memset-zero tiles so the HAM un-throttles before real work
        # arrives. The dummy PSUM tile reuses the psK tag (no extra bank).
        wdum = ewpool.tile([P, P], BF16, tag="wdum", name="wdum")
        nc.vector.memset(wdum[:], 0)
        rdum = ewpool.tile([P, SC], BF16, tag="rdum", name="rdum")
        nc.vector.memset(rdum[:], 0)
        # 5 dummies: enough to keep the PE busy from the end of the preamble
        # (~8.1us) until the first real operands land (~9.5us); the real MM
        # stream then continues the HAM warmup window.
        psdum = pspool.tile([P, SC], F32, tag="psK", bufs=3, name="psdum")
        for _ in range(5):
            nc.tensor.matmul(psdum[:], wdum[:], rdum[:], start=True, stop=True)

        x8_chunks = {}
        xb_chunks = {}

        # Input loads issue on the (otherwise idle) gpsimd queue, output
        # stores on sync: the ~0.65us per-DMA issue cost is per-engine, and
        # serializing ~16 input DMAs behind sync stalled G(0,0) by ~3.5us.
        def load_x_chunk(j):
            # one DMA per chunk & dtype: fp8 [128, ND, SC], bf16 [128, ND, SC].
            # fp8-G chunks never read bf16 slabs < N_G_F8, so skip them.
            xt = xpool.tile([P, ND, SC], F8, tag="x8b", name=f"x8b_{j}")
            nc.gpsimd.dma_start(xt[:], x8[j * P:(j + 1) * P, :, :])
            x8_chunks[j] = xt
            xtb = xpool.tile([P, ND, SC], BF16, tag="xbb", name=f"xbb_{j}")
            d0 = N_G_F8 if j < G_F8_CHUNKS else 0
            nc.gpsimd.dma_start(xtb[:, d0:, :], xb[j * P:(j + 1) * P, d0:, :])
            xb_chunks[j] = xtb

        # Startup: x chunk 0 is on the critical path to the first matmul, so
        # split it into 4 slab-pair DMAs (parallel queues + per-MM wait
        # granularity); then the i-blocked Wz/Wh tiles interleaved — K(0,i)
        # unblocks as soon as Wz[i] lands, so the PE ramps with the DMA
        # stream.
        Wz_t, Wh8_t, Whb_t, Wo_t = [], [], [], []
        xt0 = xpool.tile([P, ND, SC], F8, tag="x8b", name="x8b_0")
        nc.sync.dma_start(xt0[:, 0:2, :], x8[0:P, 0:2, :])
        wzt = wpool.tile([P, ND, P], Wz.dtype, tag="wz0", name="wz0")
        nc.gpsimd.dma_start(wzt[:], Wz[0:P, :, :])
        Wz_t.append(wzt)
        nc.sync.dma_start(xt0[:, 2:, :], x8[0:P, 2:, :])
        x8_chunks[0] = xt0

        bias_sb = wpool.tile([P, 4 * NH], F32, tag="bias", name="bias_sb")
        nc.sync.dma_start(bias_sb[:], biasT[:, :])
        bz_sb = bias_sb[:, 0:NH]
        nbz_sb = bias_sb[:, NH:2 * NH]
        bh_sb = bias_sb[:, 2 * NH:3 * NH]
        bh5_sb = bias_sb[:, 3 * NH:4 * NH]

        # G(0,0)'s operands next: Wh tiles for i=0, then chunk-0 bf16 x in
        # slab-pair DMAs so each G(0,0) matmul unblocks as its pair lands
        # (a single 1MB xtb0 DMA measured a 5.3us PE stall at t~14.5us).
        wzt = wpool.tile([P, ND, P], Wz.dtype, tag="wz1", name="wz1")
        nc.sync.dma_start(wzt[:], Wz[P:2 * P, :, :])
        Wz_t.append(wzt)

        def load_wh(i, eng):
            if N_G_F8:
                w8 = wpool.tile([P, N_G_F8, P], F8, tag=f"wh8_{i}",
                                name=f"wh8_{i}")
                eng.dma_start(w8[:], Wh8[i * P:(i + 1) * P, :, :])
                Wh8_t.append(w8)
            wbt = wpool.tile([P, ND, P], BF16, tag=f"whb_{i}", name=f"whb_{i}")
            eng.dma_start(wbt[:], Whb[i * P:(i + 1) * P, :, :])
            Whb_t.append(wbt)

        load_wh(0, nc.gpsimd)
        xtb0 = xpool.tile([P, ND, SC], BF16, tag="xbb", name="xbb_0")
        d0 = N_G_F8 if 0 < G_F8_CHUNKS else 0
        for dp in range(d0, ND, 2):
            eng = nc.sync if dp % 4 == d0 % 4 else nc.gpsimd
            eng.dma_start(xtb0[:, dp:dp + 2, :], xb[0:P, dp:dp + 2, :])
        xb_chunks[0] = xtb0
        for i in range(2, NH):
            wzt = wpool.tile([P, ND, P], Wz.dtype, tag=f"wz{i}", name=f"wz{i}")
            nc.sync.dma_start(wzt[:], Wz[i * P:(i + 1) * P, :, :])
            Wz_t.append(wzt)
            load_wh(i - 1, nc.gpsimd)
        load_wh(NH - 1, nc.gpsimd)

        def load_wo():
            for o in range(NH):
                wot = wpool.tile([P, H], BF16, tag=f"wo{o}", name=f"wo{o}")
                nc.gpsimd.dma_start(wot[:], Wo[o * P:(o + 1) * P, :])
                Wo_t.append(wot)

        h_tiles = [[None] * NH for _ in range(NSC)]

        stash = {}

        def emit_k(j, i):
            xc = x8_chunks[j]
            psK = pspool.tile([P, SC], F32, tag="psK", bufs=3,
                              name=f"psK_{j}_{i}")
            if K_F8:
                for dp in range(ND // 2):
                    nc.tensor.matmul(
                        psK[:], Wz_t[i][:, 2 * dp:2 * dp + 2, :],
                        xc[:, 2 * dp:2 * dp + 2, :],
                        start=(dp == 0), stop=(dp == ND // 2 - 1),
                        perf_mode=DR)
            else:
                for d in range(ND):
                    nc.tensor.matmul(
                        psK[:], Wz_t[i][:, d:d + 1, :], xc[:, d:d + 1, :],
                        start=(d == 0), stop=(d == ND - 1))
            A = ewpool.tile([P, SC], F32, tag="A", bufs=3, name=f"A_{j}_{i}")
            nc.scalar.activation(A[:], psK[:], AF.Sigmoid,
                                 bias=nbz_sb[:, i:i + 1], scale=-KS)
            z = ewpool.tile([P, SC], F32, tag="z", bufs=3, name=f"z_{j}_{i}")
            nc.scalar.activation(z[:], psK[:], AF.Sigmoid,
                                 bias=bz_sb[:, i:i + 1], scale=KS)
            stash[(j, i)] = (A, z)

        def emit_g(j, i):
            psG = pspool.tile([P, SC], F32, tag="psG", bufs=3,
                              name=f"psG_{j}_{i}")
            fp8j = j < G_F8_CHUNKS
            if fp8j:
                xc8 = x8_chunks[j]
                for dp in range(N_G_F8 // 2):
                    nc.tensor.matmul(
                        psG[:], Wh8_t[i][:, 2 * dp:2 * dp + 2, :],
                        xc8[:, 2 * dp:2 * dp + 2, :],
                        start=(dp == 0), stop=False, perf_mode=DR)
            xcb = xb_chunks[j]
            d0 = N_G_F8 if fp8j else 0
            for d in range(d0, ND):
                nc.tensor.matmul(
                    psG[:], Whb_t[i][:, d:d + 1, :], xcb[:, d:d + 1, :],
                    start=(d == 0 and not fp8j), stop=(d == ND - 1))
            A, z = stash.pop((j, i))
            sg = ewpool.tile([P, SC], F32, tag="sg", name=f"sg_{j}_{i}")
            nc.scalar.activation(sg[:], psG[:], AF.Sigmoid,
                                 bias=bh_sb[:, i:i + 1], scale=GS)
            g = ewpool.tile([P, SC], F32, tag="g", name=f"g_{j}_{i}")
            if N_G_F8:
                # linear branch needs the 2^-15 descale before +bh5: one
                # extra ACT op (Identity has working bias+scale ports), then
                # the max moves to a plain DVE tensor_tensor.
                t = ewpool.tile([P, SC], F32, tag="t", name=f"t_{j}_{i}")
                nc.scalar.activation(t[:], psG[:], AF.Identity,
                                     bias=bh5_sb[:, i:i + 1], scale=GS)
                nc.vector.tensor_tensor(g[:], t[:], sg[:], op=OP.max)
            else:
                nc.vector.scalar_tensor_tensor(g[:], psG[:], bh5_sb[:, i:i + 1],
                                               sg[:], op0=OP.add, op1=OP.max)
            Bv = ewpool.tile([P, SC], F32, tag="B", name=f"B_{j}_{i}")
            nc.vector.tensor_tensor(Bv[:], z[:], g[:], op=OP.mult)

            ht = hpool.tile([P, SC], BF16, tag=f"h{i}", name=f"h_{j}_{i}")
            init = 0.0 if j == 0 else h_tiles[j - 1][i][:, SC - 1:SC]
            nc.vector.tensor_tensor_scan(ht[:], A[:], Bv[:], initial=init,
                                         op0=OP.mult, op1=OP.add)
            h_tiles[j][i] = ht

        def emit_o(j, o):
            psO = pspool.tile([P, SC], F32, tag="psO", name=f"psO_{j}_{o}")
            for i in range(NH):
                nc.tensor.matmul(
                    psO[:], Wo_t[o][:, i * P:(i + 1) * P],
                    h_tiles[j][i][:],
                    start=(i == 0), stop=(i == NH - 1))
            oc = opool.tile([P, SC], F32, tag="oc", name=f"oc_{j}_{o}")
            nc.scalar.copy(oc[:], psO[:])
            nc.sync.dma_start(outT[o * P:(o + 1) * P, j * SC:(j + 1) * SC], oc[:])

        # Software pipeline. Per chunk j the PE group order is
        #   K0 K1 [G0 O0] [K2 G1 O1] [K3 G2 O2] ... [K7 G6 O6] [G7 O7]
        # where O* are the GEMM3 groups of chunk j-1. Interleaving the O
        # groups keeps ~2 PE groups between G(i) and the DVE/ACT chain that
        # releases its PSUM bank, so the PE never stalls on the elementwise
        # tail. x(j+1) is prefetched at the head of chunk j; Wo loads are
        # issued at the head of chunk 1 (first needed by GEMM3 of chunk 0).
        for j in range(NSC):
            if j + 1 < NSC:
                load_x_chunk(j + 1)
            if j == 1:
                load_wo()
            # chunk 0 leads with 4 K groups (vs 2): buys the DMA stream an
            # extra ~1.7us before G(0,0)'s weights/x are needed
            lead = 4 if j == 0 else 2
            for i in range(lead):
                emit_k(j, i)
            for i in range(NH):
                if i + lead < NH:
                    emit_k(j, i + lead)
                emit_g(j, i)
                if j >= 1:
                    emit_o(j - 1, i)
        for o in range(NH - 1):
            emit_o(NSC - 1, o)
        # final O group split into two N=256 halves so the first half's
        # copy+store overlaps the second half's matmuls (shorter serial
        # tail before the drain barrier); PSUM/SBUF tags are reused so no
        # extra banks are allocated
        j, o = NSC - 1, NH - 1
        HC = SC // 2
        for half in range(2):
            psO = pspool.tile([P, HC], F32, tag="psO", name=f"psOt_{half}")
            for i in range(NH):
                nc.tensor.matmul(
                    psO[:], Wo_t[o][:, i * P:(i + 1) * P],
                    h_tiles[j][i][:, half * HC:(half + 1) * HC],
                    start=(i == 0), stop=(i == NH - 1))
            oc = opool.tile([P, HC], F32, tag="oc", name=f"oct_{half}")
            nc.scalar.copy(oc[:], psO[:])
            nc.sync.dma_start(
                outT[o * P:(o + 1) * P,
                     j * SC + half * HC:j * SC + (half + 1) * HC], oc[:])

    nc.compile()
    return nc


_CACHE = {}


def _get_module():
    if "nc" not in _CACHE:
        _CACHE["nc"] = _build_module()
    return _CACHE["nc"]


def _make_in_maps(x, Wz_f, bz_f, Wh_f, bh_f, Wz_b, bz_b, Wh_b, bh_b, W_out, b_out):
    np_f8 = _np_f8()
    np_bf = _np_bf16()
    f32 = np.float32

    def q8(a):
        # host fp8-e4m3 quantization; TRN FP8_EXP4 max normal is 240
        return np.clip(a, -240.0, 240.0).astype(np_f8)

    def blk_w(w):
        # [D, H] -> blocked [H, ND, P]: out[i*128+p, d, c] = w[d*128+p, i*128+c]
        w = np.asarray(w, dtype=f32).reshape(ND, P, NH, P)
        return np.ascontiguousarray(w.transpose(2, 1, 0, 3).reshape(H, ND, P))

    def blk_x(xs, rev):
        # [S, D] -> blocked [NSC*P, ND, SC]: out[j*128+p, d, c] = x[j*512+c, d*128+p]
        if rev:
            xs = xs[::-1]
        xs = xs.reshape(NSC, SC, ND, P)
        return np.ascontiguousarray(
            xs.transpose(0, 3, 2, 1).reshape(NSC * P, ND, SC))

    x = np.asarray(x, dtype=f32)
    W_out = np.asarray(W_out)

    def w_maps(Wz, Wh, W_half):
        m = {}
        wzb = blk_w(np.asarray(Wz, f32))
        m["Wz"] = q8(wzb * SW) if K_F8 else wzb.astype(np_bf)
        whb = blk_w(np.asarray(Wh, f32))
        if N_G_F8:
            m["Wh8"] = q8(whb[:, :N_G_F8] * SW)
        m["Whb"] = np.ascontiguousarray(whb * (PS if N_G_F8 else 1.0)
                                        ).astype(np_bf)
        wo = np.asarray(W_half, f32).reshape(NH, P, NH, P)
        m["Wo"] = np.ascontiguousarray(
            wo.transpose(2, 1, 0, 3).reshape(H, H)).astype(np_bf)
        return m

    wm_f = w_maps(Wz_f, Wh_f, W_out[:H])
    wm_b = w_maps(Wz_b, Wh_b, W_out[H:])

    def bias_pack(b_z, b_h):
        def col(v):  # [H] -> [128, NH] with col i = h-tile i
            return np.asarray(v, dtype=f32).reshape(NH, P).T
        b_z = np.asarray(b_z, dtype=f32)
        b_h = np.asarray(b_h, dtype=f32)
        return np.ascontiguousarray(np.concatenate(
            [col(b_z), col(-b_z), col(b_h), col(b_h + 0.5)], axis=1))

    bias_f = bias_pack(bz_f, bh_f)
    bias_b = bias_pack(bz_b, bh_b)

    in_maps = []
    for b in range(4):
        for rev, wm, bm in ((False, wm_f, bias_f), (True, wm_b, bias_b)):
            xblk = blk_x(x[b], rev=rev)
            in_maps.append({
                "x8": q8(xblk * SX),
                "xb": xblk.astype(np_bf),
                "biasT": bm, **wm})
    return in_maps


def _assemble(results, b_out):
    out = np.empty((4, S, H), np.float32)
    for b in range(4):
        out[b] = results[2 * b]["outT"].T
        out[b] += results[2 * b + 1]["outT"].T
    out += np.asarray(b_out, dtype=np.float32)
    return out


def kernel(x, Wz_f, bz_f, Wh_f, bh_f, Wz_b, bz_b, Wh_b, bh_b, W_out, b_out):
    nc = _get_module()
    in_maps = _make_in_maps(x, Wz_f, bz_f, Wh_f, bh_f,
                            Wz_b, bz_b, Wh_b, bh_b, W_out, b_out)
    res = run_bass_kernel_spmd(nc, in_maps, core_ids=list(range(NCORES)))
    return _assemble(res.results, b_out)


# revision 10
# speedup vs baseline: 1.1975x; 1.0005x over previous
"""Bi-directional MinGRU kernel for Trainium2 (8 NeuronCores, SPMD).

Problem: x [4, 4096, 1024]; per direction d in {fwd, bwd}:
    k  = x @ Wz_d + bz_d
    A  = sigmoid(-k)           (= 1 - z, the carry coefficient)
    z  = sigmoid(k)
    gp = x @ Wh_d + bh_d
    g  = max(gp + 0.5, sigmoid(gp))      (== where(gp>=0, gp+0.5, sigmoid(gp)))
    h_t = A_t * h_{t-1} + z_t * g_t      (linear first-order scan over S)
    out = concat(h_fwd, h_bwd) @ W_out + b_out

Sharding: 8 cores = (4 batches) x (2 directions). Each core computes the
full hidden state for one (batch, direction) and its half of the final
2H->H projection; the two partial products per batch are summed on host.

Per-core layout: everything is kept transposed ([channel, seq]) so the
sequential scan runs along the free dimension with channels on partitions,
using the native VectorE tensor_tensor_scan instruction.

Mixed precision: the K-GEMM (gate pre-activation k) runs entirely in
fp8-e4m3 with perf_mode=DoubleRow (2 contraction slabs per matmul, 0.5
cyc/row): its error is damped ~4x by the sigmoids. The G-GEMM runs
N_G_F8 of its 8 contraction slabs in fp8-DR (the g path passes error
1:1 where gp>0, so only a fraction fits in the error budget). The
O-GEMM stays bf16 (direct output path, most sensitive). fp8 operands
are pre-scaled on host (x*32, W*1024 -> PSUM = 2^15 * logical value,
clipped to +-240 = TRN FP8_EXP4 max normal); the 2^-15 descale folds
into the ACT scale ports. bf16 G-slabs' weights are pre-scaled 2^15 so
both halves accumulate into PSUM at the same scale (power-of-2 scaling
is exact in bf16).

Measured (2026-08-08): rel 1.597e-2 (sim predicted 1.606e-2 — DR is
bit-correct on HW). HW exec 340.3us, but the WHOLE CORE clocked at
~2.0GHz instead of ~2.4GHz during the run (all engines' op durations
uniformly 1.2x the baseline run's) — investigating whether that's
kernel-induced (power) or environment drift.
"""

import os
import numpy as np
from contextlib import ExitStack

import concourse.bass as bass
import concourse.tile as tile
from concourse import bacc, mybir
from concourse.bass_utils import run_bass_kernel_spmd

P = 128          # partitions
S = 4096         # sequence length
D = 1024         # input dim
H = 1024         # hidden dim
SC = 512         # seq chunk (one PSUM bank of fp32)
NSC = S // SC    # 8 seq chunks
ND = D // P      # 8 contraction tiles for GEMM1
NH = H // P      # 8 hidden tiles
NCORES = 8

F32 = mybir.dt.float32
BF16 = mybir.dt.bfloat16
F8 = mybir.dt.float8e4

# fp8 slab counts (build-time constants): K-GEMM all-fp8, G-GEMM N_G_F8
# of 8 slabs fp8 (rest bf16) on the first G_F8_CHUNKS chunks only, O-GEMM
# bf16. G_F8_CHUNKS trades precision for DR-matmul density: the SoC power
# manager drops the whole-core clock 2.4->2.0GHz when the average DR
# density crosses a threshold somewhere in (20%, 26.5%) of matmul count
# (measured: K-only = 256 DR of 1288 runs at full clock; K+G2-all-chunks
# = 320 of 1209 throttles, erasing the fp8 gain).
K_F8 = os.environ.get("BIMINGRU_K_F8", "1") == "1"
N_G_F8 = int(os.environ.get("BIMINGRU_G_F8", "2"))
# 6 chunks = 304 DR of ~1245 matmuls = 24.4% DR density: the highest
# measured config that holds the full 2.4GHz clock (312/1237 = 25.2%
# throttles to 2.0GHz; the threshold sits at 25%).
G_F8_CHUNKS = int(os.environ.get("BIMINGRU_G_F8_CHUNKS", "6"))
if N_G_F8 == 0:
    G_F8_CHUNKS = 0
if G_F8_CHUNKS == 0:
    N_G_F8 = 0
assert N_G_F8 % 2 == 0 and 0 <= N_G_F8 <= ND
NGB = ND - N_G_F8                # bf16 G slabs on fp8-G chunks

SX = 32.0                        # host scale on fp8 x
SW = 1024.0                      # host scale on fp8 W
PS = SX * SW                     # PSUM scale of fp8-fed GEMMs (2^15)
KS = (1.0 / PS) if K_F8 else 1.0     # K descale folded into ACT scale
GS = (1.0 / PS) if N_G_F8 else 1.0   # G descale

DR = mybir.MatmulPerfMode.DoubleRow


def _np_f8():
    import ml_dtypes
    return np.dtype(ml_dtypes.float8_e4m3)


def _np_bf16():
    import ml_dtypes
    return np.dtype(ml_dtypes.bfloat16)


def _build_module():
    nc = bacc.Bacc("TRN2", target_bir_lowering=False, debug=False)

    # All inputs are host-blocked so every SBUF working set is ONE contiguous
    # DMA (the sync engine's ~0.65us per-DMA issue cost dominates the ramp):
    #   xT8 [NSC*P, ND, SC]: [j*128+p, d, c] = 32*x[j*512+c, d*128+p] (fp8)
    #   xTb [NSC*P, NGB, SC]: same but bf16, slabs d >= N_G_F8 only, unscaled
    #   Wz8 [NH*P, ND, P]:  [i*128+p, d, c] = 1024*Wz[d*128+p, i*128+c] (fp8)
    #   Wh8 [NH*P, N_G_F8, P]: fp8 slabs of Wh (scaled 1024)
    #   Whb [NH*P, NGB, P]: bf16 slabs of Wh (scaled 2^15 iff N_G_F8>0)
    #   Wo  [H, H] bf16: [o*128+p, i*128+c] = W_half[i*128+p, o*128+c]
    #   biasT [128, 4*NH] = [bz | -bz | bh | bh+0.5] per-partition columns
    x8 = nc.dram_tensor("x8", [NSC * P, ND, SC], F8, kind="ExternalInput").ap()
    if K_F8:
        Wz = nc.dram_tensor("Wz", [NH * P, ND, P], F8, kind="ExternalInput").ap()
    else:
        Wz = nc.dram_tensor("Wz", [NH * P, ND, P], BF16, kind="ExternalInput").ap()
    if N_G_F8:
        Wh8 = nc.dram_tensor("Wh8", [NH * P, N_G_F8, P], F8,
                             kind="ExternalInput").ap()
    xb = nc.dram_tensor("xb", [NSC * P, ND, SC], BF16, kind="ExternalInput").ap()
    Whb = nc.dram_tensor("Whb", [NH * P, ND, P], BF16, kind="ExternalInput").ap()
    Wo = nc.dram_tensor("Wo", [H, H], BF16, kind="ExternalInput").ap()
    biasT = nc.dram_tensor("biasT", [P, 4 * NH], F32, kind="ExternalInput").ap()
    outT = nc.dram_tensor("outT", [H, S], F32, kind="ExternalOutput").ap()

    AF = mybir.ActivationFunctionType
    OP = mybir.AluOpType

    with tile.TileContext(nc) as tc, ExitStack() as ctx:
        wpool = ctx.enter_context(tc.tile_pool(name="w", bufs=1))
        xpool = ctx.enter_context(tc.tile_pool(name="x", bufs=2))
        pspool = ctx.enter_context(tc.tile_pool(name="ps", bufs=2, space="PSUM"))
        ewpool = ctx.enter_context(tc.tile_pool(name="ew", bufs=2))
        hpool = ctx.enter_context(tc.tile_pool(name="h", bufs=2))
        opool = ctx.enter_context(tc.tile_pool(name="o", bufs=3))

        # --- PE warm-up: the first real matmul can't start until the head of
        # the input DMA stream lands, and a cold PE then runs at 1.2GHz for
        # another ~3.4us (HAM). Burn that idle window with dummy matmuls on
        # memset-zero tiles so the HAM un-throttles before real work
        # arrives. The dummy PSUM tile reuses the psK tag (no extra bank).
        wdum = ewpool.tile([P, P], BF16, tag="wdum", name="wdum")
        nc.vector.memset(wdum[:], 0)
        rdum = ewpool.tile([P, SC], BF16, tag="rdum", name="rdum")
        nc.vector.memset(rdum[:], 0)
        # 5 dummies: enough to keep the PE busy from the end of the preamble
        # (~8.1us) until the first real operands land (~9.5us); the real MM
        # stream then continues the HAM warmup window.
        psdum = pspool.tile([P, SC], F32, tag="psK", bufs=3, name="psdum")
        for _ in range(5):
            nc.tensor.matmul(psdum[:], wdum[:], rdum[:], start=True, stop=True)

        x8_chunks = {}
        xb_chunks = {}

        # Input loads issue on the (otherwise idle) gpsimd queue, output
        # stores on sync: the ~0.65us per-DMA issue cost is per-engine, and
        # serializing ~16 input DMAs behind sync stalled G(0,0) by ~3.5us.
        def load_x_chunk(j):
            # one DMA per chunk & dtype: fp8 [128, ND, SC], bf16 [128, ND, SC].
            # fp8-G chunks never read bf16 slabs < N_G_F8, so skip them.
            xt = xpool.tile([P, ND, SC], F8, tag="x8b", name=f"x8b_{j}")
            nc.gpsimd.dma_start(xt[:], x8[j * P:(j + 1) * P, :, :])
            x8_chunks[j] = xt
            xtb = xpool.tile([P, ND, SC], BF16, tag="xbb", name=f"xbb_{j}")
            d0 = N_G_F8 if j < G_F8_CHUNKS else 0
            nc.gpsimd.dma_start(xtb[:, d0:, :], xb[j * P:(j + 1) * P, d0:, :])
            xb_chunks[j] = xtb

        # Startup: x chunk 0 is on the critical path to the first matmul, so
        # split it into 4 slab-pair DMAs (parallel queues + per-MM wait
        # granularity); then the i-blocked Wz/Wh tiles interleaved — K(0,i)
        # unblocks as soon as Wz[i] lands, so the PE ramps with the DMA
        # stream.
        Wz_t, Wh8_t, Whb_t, Wo_t = [], [], [], []
        xt0 = xpool.tile([P, ND, SC], F8, tag="x8b", name="x8b_0")
        nc.sync.dma_start(xt0[:, 0:2, :], x8[0:P, 0:2, :])
        wzt = wpool.tile([P, ND, P], Wz.dtype, tag="wz0", name="wz0")
        nc.gpsimd.dma_start(wzt[:], Wz[0:P, :, :])
        Wz_t.append(wzt)
        nc.sync.dma_start(xt0[:, 2:, :], x8[0:P, 2:, :])
        x8_chunks[0] = xt0

        bias_sb = wpool.tile([P, 4 * NH], F32, tag="bias", name="bias_sb")
        nc.sync.dma_start(bias_sb[:], biasT[:, :])
        bz_sb = bias_sb[:, 0:NH]
        nbz_sb = bias_sb[:, NH:2 * NH]
        bh_sb = bias_sb[:, 2 * NH:3 * NH]
        bh5_sb = bias_sb[:, 3 * NH:4 * NH]

        # G(0,0)'s operands next: Wh tiles for i=0, then chunk-0 bf16 x in
        # slab-pair DMAs so each G(0,0) matmul unblocks as its pair lands
        # (a single 1MB xtb0 DMA measured a 5.3us PE stall at t~14.5us).
        wzt = wpool.tile([P, ND, P], Wz.dtype, tag="wz1", name="wz1")
        nc.sync.dma_start(wzt[:], Wz[P:2 * P, :, :])
        Wz_t.append(wzt)

        def load_wh(i, eng):
            # startup loads only the slabs the fp8-G chunks read (>= N_G_F8);
            # slabs [0, N_G_F8) are first needed by chunk G_F8_CHUNKS and are
            # deferred to the head of chunk 2 (off the startup bandwidth peak)
            if N_G_F8:
                w8 = wpool.tile([P, N_G_F8, P], F8, tag=f"wh8_{i}",
                                name=f"wh8_{i}")
                eng.dma_start(w8[:], Wh8[i * P:(i + 1) * P, :, :])
                Wh8_t.append(w8)
            wbt = wpool.tile([P, ND, P], BF16, tag=f"whb_{i}", name=f"whb_{i}")
            d0 = N_G_F8 if G_F8_CHUNKS > 0 else 0
            eng.dma_start(wbt[:, d0:, :], Whb[i * P:(i + 1) * P, d0:, :])
            Whb_t.append(wbt)

        def load_wh_low():
            d0 = N_G_F8 if G_F8_CHUNKS > 0 else 0
            if d0 and G_F8_CHUNKS < NSC:
                for i in range(NH):
                    nc.gpsimd.dma_start(Whb_t[i][:, :d0, :],
                                        Whb[i * P:(i + 1) * P, :d0, :])

        load_wh(0, nc.gpsimd)
        xtb0 = xpool.tile([P, ND, SC], BF16, tag="xbb", name="xbb_0")
        d0 = N_G_F8 if 0 < G_F8_CHUNKS else 0
        for dp in range(d0, ND, 2):
            eng = nc.sync if dp % 4 == d0 % 4 else nc.gpsimd
            eng.dma_start(xtb0[:, dp:dp + 2, :], xb[0:P, dp:dp + 2, :])
        xb_chunks[0] = xtb0
        for i in range(2, NH):
            wzt = wpool.tile([P, ND, P], Wz.dtype, tag=f"wz{i}", name=f"wz{i}")
            nc.sync.dma_start(wzt[:], Wz[i * P:(i + 1) * P, :, :])
            Wz_t.append(wzt)
            load_wh(i - 1, nc.gpsimd)
        load_wh(NH - 1, nc.gpsimd)

        def load_wo():
            for o in range(NH):
                wot = wpool.tile([P, H], BF16, tag=f"wo{o}", name=f"wo{o}")
                nc.gpsimd.dma_start(wot[:], Wo[o * P:(o + 1) * P, :])
                Wo_t.append(wot)

        h_tiles = [[None] * NH for _ in range(NSC)]

        stash = {}

        def emit_k(j, i):
            xc = x8_chunks[j]
            psK = pspool.tile([P, SC], F32, tag="psK", bufs=3,
                              name=f"psK_{j}_{i}")
            if K_F8:
                for dp in range(ND // 2):
                    nc.tensor.matmul(
                        psK[:], Wz_t[i][:, 2 * dp:2 * dp + 2, :],
                        xc[:, 2 * dp:2 * dp + 2, :],
                        start=(dp == 0), stop=(dp == ND // 2 - 1),
                        perf_mode=DR)
            else:
                for d in range(ND):
                    nc.tensor.matmul(
                        psK[:], Wz_t[i][:, d:d + 1, :], xc[:, d:d + 1, :],
                        start=(d == 0), stop=(d == ND - 1))
            A = ewpool.tile([P, SC], F32, tag="A", bufs=3, name=f"A_{j}_{i}")
            nc.scalar.activation(A[:], psK[:], AF.Sigmoid,
                                 bias=nbz_sb[:, i:i + 1], scale=-KS)
            z = ewpool.tile([P, SC], F32, tag="z", bufs=3, name=f"z_{j}_{i}")
            nc.scalar.activation(z[:], psK[:], AF.Sigmoid,
                                 bias=bz_sb[:, i:i + 1], scale=KS)
            stash[(j, i)] = (A, z)

        def emit_g(j, i):
            psG = pspool.tile([P, SC], F32, tag="psG", bufs=3,
                              name=f"psG_{j}_{i}")
            fp8j = j < G_F8_CHUNKS
            if fp8j:
                xc8 = x8_chunks[j]
                for dp in range(N_G_F8 // 2):
                    nc.tensor.matmul(
                        psG[:], Wh8_t[i][:, 2 * dp:2 * dp + 2, :],
                        xc8[:, 2 * dp:2 * dp + 2, :],
                        start=(dp == 0), stop=False, perf_mode=DR)
            xcb = xb_chunks[j]
            d0 = N_G_F8 if fp8j else 0
            for d in range(d0, ND):
                nc.tensor.matmul(
                    psG[:], Whb_t[i][:, d:d + 1, :], xcb[:, d:d + 1, :],
                    start=(d == 0 and not fp8j), stop=(d == ND - 1))
            A, z = stash.pop((j, i))
            sg = ewpool.tile([P, SC], F32, tag="sg", name=f"sg_{j}_{i}")
            nc.scalar.activation(sg[:], psG[:], AF.Sigmoid,
                                 bias=bh_sb[:, i:i + 1], scale=GS)
            g = ewpool.tile([P, SC], F32, tag="g", name=f"g_{j}_{i}")
            if N_G_F8:
                # linear branch needs the 2^-15 descale before +bh5: one
                # extra ACT op (Identity has working bias+scale ports), then
                # the max moves to a plain DVE tensor_tensor.
                t = ewpool.tile([P, SC], F32, tag="t", name=f"t_{j}_{i}")
                nc.scalar.activation(t[:], psG[:], AF.Identity,
                                     bias=bh5_sb[:, i:i + 1], scale=GS)
                nc.vector.tensor_tensor(g[:], t[:], sg[:], op=OP.max)
            else:
                nc.vector.scalar_tensor_tensor(g[:], psG[:], bh5_sb[:, i:i + 1],
                                               sg[:], op0=OP.add, op1=OP.max)
            Bv = ewpool.tile([P, SC], F32, tag="B", name=f"B_{j}_{i}")
            nc.vector.tensor_tensor(Bv[:], z[:], g[:], op=OP.mult)

            ht = hpool.tile([P, SC], BF16, tag=f"h{i}", name=f"h_{j}_{i}")
            init = 0.0 if j == 0 else h_tiles[j - 1][i][:, SC - 1:SC]
            nc.vector.tensor_tensor_scan(ht[:], A[:], Bv[:], initial=init,
                                         op0=OP.mult, op1=OP.add)
            h_tiles[j][i] = ht

        def emit_o(j, o):
            psO = pspool.tile([P, SC], F32, tag="psO", name=f"psO_{j}_{o}")
            for i in range(NH):
                nc.tensor.matmul(
                    psO[:], Wo_t[o][:, i * P:(i + 1) * P],
                    h_tiles[j][i][:],
                    start=(i == 0), stop=(i == NH - 1))
            oc = opool.tile([P, SC], F32, tag="oc", name=f"oc_{j}_{o}")
            nc.scalar.copy(oc[:], psO[:])
            nc.sync.dma_start(outT[o * P:(o + 1) * P, j * SC:(j + 1) * SC], oc[:])

        # Software pipeline. Per chunk j the PE group order is
        #   K0 K1 [G0 O0] [K2 G1 O1] [K3 G2 O2] ... [K7 G6 O6] [G7 O7]
        # where O* are the GEMM3 groups of chunk j-1. Interleaving the O
        # groups keeps ~2 PE groups between G(i) and the DVE/ACT chain that
        # releases its PSUM bank, so the PE never stalls on the elementwise
        # tail. x(j+1) is prefetched at the head of chunk j; Wo loads are
        # issued at the head of chunk 1 (first needed by GEMM3 of chunk 0).
        for j in range(NSC):
            if j == 1:
                load_wo()
            if j == 2:
                load_wh_low()
            # chunk 0 leads with 4 K groups (vs 2): buys the DMA stream an
            # extra ~1.7us before G(0,0)'s weights/x are needed
            lead = 4 if j == 0 else 2
            for i in range(lead):
                emit_k(j, i)
            for i in range(NH):
                if i + lead < NH:
                    emit_k(j, i + lead)
                emit_g(j, i)
                # next chunk's x prefetch mid-chunk: off the startup/Wo
                # bandwidth peaks, still ~5 groups (~4us) of transfer slack
                if i == 2 and j + 1 < NSC:
                    load_x_chunk(j + 1)
                if j >= 1:
                    emit_o(j - 1, i)
        for o in range(NH - 1):
            emit_o(NSC - 1, o)
        # final O group split into two N=256 halves so the first half's
        # copy+store overlaps the second half's matmuls (shorter serial
        # tail before the drain barrier); PSUM/SBUF tags are reused so no
        # extra banks are allocated
        j, o = NSC - 1, NH - 1
        HC = SC // 2
        for half in range(2):
            psO = pspool.tile([P, HC], F32, tag="psO", name=f"psOt_{half}")
            for i in range(NH):
                nc.tensor.matmul(
                    psO[:], Wo_t[o][:, i * P:(i + 1) * P],
                    h_tiles[j][i][:, half * HC:(half + 1) * HC],
                    start=(i == 0), stop=(i == NH - 1))
            oc = opool.tile([P, HC], F32, tag="oc", name=f"oct_{half}")
            nc.scalar.copy(oc[:], psO[:])
            nc.sync.dma_start(
                outT[o * P:(o + 1) * P,
                     j * SC + half * HC:j * SC + (half + 1) * HC], oc[:])

    nc.compile()
    return nc


_CACHE = {}


def _get_module():
    if "nc" not in _CACHE:
        _CACHE["nc"] = _build_module()
    return _CACHE["nc"]


def _make_in_maps(x, Wz_f, bz_f, Wh_f, bh_f, Wz_b, bz_b, Wh_b, bh_b, W_out, b_out):
    np_f8 = _np_f8()
    np_bf = _np_bf16()
    f32 = np.float32

    def q8(a):
        # host fp8-e4m3 quantization; TRN FP8_EXP4 max normal is 240
        return np.clip(a, -240.0, 240.0).astype(np_f8)

    def blk_w(w):
        # [D, H] -> blocked [H, ND, P]: out[i*128+p, d, c] = w[d*128+p, i*128+c]
        w = np.asarray(w, dtype=f32).reshape(ND, P, NH, P)
        return np.ascontiguousarray(w.transpose(2, 1, 0, 3).reshape(H, ND, P))

    def blk_x(xs, rev):
        # [S, D] -> blocked [NSC*P, ND, SC]: out[j*128+p, d, c] = x[j*512+c, d*128+p]
        if rev:
            xs = xs[::-1]
        xs = xs.reshape(NSC, SC, ND, P)
        return np.ascontiguousarray(
            xs.transpose(0, 3, 2, 1).reshape(NSC * P, ND, SC))

    x = np.asarray(x, dtype=f32)
    W_out = np.asarray(W_out)

    def w_maps(Wz, Wh, W_half):
        m = {}
        wzb = blk_w(np.asarray(Wz, f32))
        m["Wz"] = q8(wzb * SW) if K_F8 else wzb.astype(np_bf)
        whb = blk_w(np.asarray(Wh, f32))
        if N_G_F8:
            m["Wh8"] = q8(whb[:, :N_G_F8] * SW)
        m["Whb"] = np.ascontiguousarray(whb * (PS if N_G_F8 else 1.0)
                                        ).astype(np_bf)
        wo = np.asarray(W_half, f32).reshape(NH, P, NH, P)
        m["Wo"] = np.ascontiguousarray(
            wo.transpose(2, 1, 0, 3).reshape(H, H)).astype(np_bf)
        return m

    wm_f = w_maps(Wz_f, Wh_f, W_out[:H])
    wm_b = w_maps(Wz_b, Wh_b, W_out[H:])

    def bias_pack(b_z, b_h):
        def col(v):  # [H] -> [128, NH] with col i = h-tile i
            return np.asarray(v, dtype=f32).reshape(NH, P).T
        b_z = np.asarray(b_z, dtype=f32)
        b_h = np.asarray(b_h, dtype=f32)
        return np.ascontiguousarray(np.concatenate(
            [col(b_z), col(-b_z), col(b_h), col(b_h + 0.5)], axis=1))

    bias_f = bias_pack(bz_f, bh_f)
    bias_b = bias_pack(bz_b, bh_b)

    in_maps = []
    for b in range(4):
        for rev, wm, bm in ((False, wm_f, bias_f), (True, wm_b, bias_b)):
            xblk = blk_x(x[b], rev=rev)
            in_maps.append({
                "x8": q8(xblk * SX),
                "xb": xblk.astype(np_bf),
                "biasT": bm, **wm})
    return in_maps


def _assemble(results, b_out):
    out = np.empty((4, S, H), np.float32)
    for b in range(4):
        out[b] = results[2 * b]["outT"].T
        out[b] += results[2 * b + 1]["outT"].T
    out += np.asarray(b_out, dtype=np.float32)
    return out


def kernel(x, Wz_f, bz_f, Wh_f, bh_f, Wz_b, bz_b, Wh_b, bh_b, W_out, b_out):
    nc = _get_module()
    in_maps = _make_in_maps(x, Wz_f, bz_f, Wh_f, bh_f,
                            Wz_b, bz_b, Wh_b, bh_b, W_out, b_out)
    res = run_bass_kernel_spmd(nc, in_maps, core_ids=list(range(NCORES)))
    return _assemble(res.results, b_out)
